# revision 22
# baseline (speedup 1.0000x reference)
"""AttentiveRNNLanguageModel Trainium2 kernel (8-core SPMD), v2.

Sharding: recurrence replicated on all 8 cores; tied embedding/decoder matmul
sharded vocab-wise 8 ways. No collectives.

v2 restructure vs baseline (32-step blocks, bf16 weights; fp8 Whh was tried
and reverted — LDWEIGHTS measured slower for fp8 than bf16 on this part):
- Main LSTM cell: xw injected into the PSUM accumulation group via an
  identity matmul so activations read PSUM directly; h written once to a
  static h16 tile (matmul operands must be static slices — dynamic ds()
  operands cost a per-matmul address-register instruction on the PE queue),
  then one dynamic copy into the padded encT; gate chunks issued g~ first,
  then i,f, then o, with split sigmoid/tanh so the cell chain overlaps the
  o-gate matmuls.
- Positional LSTM trails the main LSTM by one block: bias + batched input
  matmuls (Wp_ih @ h, N=128 moving) pre-seed a per-block PSUM bank that the
  per-step Wp_hh matmuls accumulate into; a zero pad block keeps the For_i
  body branch-free, and the last block runs in a Python epilogue.
- mw/sigma/mu/den per-step chains removed from the loop: hp is stacked, one
  batched W3 matmul per block + relu/sigmoid produce a/b/sigma stacks, and the
  mu recurrence runs post-loop as 4 tensor_tensor_scan instructions.
- xw round-trip through DRAM in bf16; two explicit SBUF tiles (xwA/xwB) with
  the prefetch DMA issued right after each tile's last reader, so the strided
  gather (~2.5us) overlaps the other block's compute instead of stalling the
  inject matmul at every block boundary (was ~7us/block + it re-throttled the
  PE clock gate for the rest of the block).
Only Sigmoid/Tanh inside the loop (one ACT table set); Exp appears once in the
post phases.
"""
import os
import numpy as np
import ml_dtypes
from contextlib import ExitStack

import concourse.bass as bass
import concourse.tile as tile
from concourse import bacc, mybir
from concourse.bass_utils import run_bass_kernel_spmd

F32 = mybir.dt.float32
BF16 = mybir.dt.bfloat16
FP8 = mybir.dt.float8e4
AF = mybir.ActivationFunctionType

B, T, H, P, V = 4, 1024, 512, 20, 32000
NCORES = 8
VSH = V // NCORES
EPS_SIG = 0.001
EPS_NORM = 1e-12
NBLK, SPB = 32, 32

LAST_EXEC_NS = [None]


def _bf(x):
    return np.ascontiguousarray(np.asarray(x).astype(ml_dtypes.bfloat16))


def _f8(x):
    return np.ascontiguousarray(np.asarray(x).astype(ml_dtypes.float8_e4m3fn))


def _f32(x):
    return np.ascontiguousarray(np.asarray(x), dtype=np.float32)


def build_nc():
    nc = bacc.Bacc()
    dt = nc.dram_tensor
    xT_in = dt("xT", [128, 4 * B * T], BF16, kind="ExternalInput")
    wihT_in = dt("wihT", [128, 4 * 16 * 128], BF16, kind="ExternalInput")
    whhT_in = dt("whhT", [128, 4 * 16 * 128], BF16, kind="ExternalInput")
    mbias_in = dt("mbias", [128, 16], F32, kind="ExternalInput")
    wpihT_in = dt("wpihT", [128, 4 * 4 * P], BF16, kind="ExternalInput")
    wphhT_in = dt("wphhT", [P, 4 * P], BF16, kind="ExternalInput")
    w3T_in = dt("w3T", [P, 4], BF16, kind="ExternalInput")
    bp80_in = dt("bp80", [1, 80], BF16, kind="ExternalInput")
    bm_in = dt("bm", [1, 16], F32, kind="ExternalInput")
    invLp_in = dt("invLp", [1, 128], F32, kind="ExternalInput")
    jl4_in = dt("jl4", [1, 4 * (32 + T)], F32, kind="ExternalInput")
    relM_in = dt("relM", [128, 8 * T], BF16, kind="ExternalInput")
    wcT_in = dt("wcT", [128, 8 * 4 * 128], BF16, kind="ExternalInput")
    bc_in = dt("bc", [128, 4], F32, kind="ExternalInput")
    embT_in = dt("embT", [128, 4 * VSH], BF16, kind="ExternalInput")
    logits_out = dt("logits", [B * T, VSH], BF16, kind="ExternalOutput")
    xwt = dt("xwt", [128, (T + 2 * SPB) * 64], BF16, kind="Internal")

    NU = 32 + T            # encT u-slots (1 pad block + T steps)
    NPS = 33 + T           # hpstack slots (33 pad + T)
    NST = 32 + T           # stack slots (32 pad + T)

    with tile.TileContext(nc) as tc, ExitStack() as ctx:
        live = ctx.enter_context(tc.tile_pool(name="live", bufs=1))
        encTbuf = live.tile([128, 16 * NU], BF16)
        mustack = live.tile([128, 4 * T], BF16)
        denstack = live.tile([128, 4 * T], BF16)
        nc.vector.memset(encTbuf[:, 0:16 * SPB], 0.0)

        # ================= Phase 1: bulk xw^T (-> DRAM bf16) ================
        with ExitStack() as p1:
            p1w = p1.enter_context(tc.tile_pool(name="p1w", bufs=1))
            p1e = p1.enter_context(tc.tile_pool(name="p1e", bufs=4))
            p1ps = p1.enter_context(tc.tile_pool(name="p1ps", bufs=6, space="PSUM"))
            xT_sb = p1w.tile([128, 4 * B * T], BF16)
            nc.sync.dma_start(xT_sb[:], xT_in[:, :])
            wih_sb = p1w.tile([128, 4 * 16 * 128], BF16)
            nc.sync.dma_start(wih_sb[:], wihT_in[:, :])
            mb_sb = p1w.tile([128, 16], F32)
            nc.sync.dma_start(mb_sb[:], mbias_in[:, :])
            for mc in range(16):
                for h2 in range(2):
                    pss = [p1ps.tile([128, 512], F32, tag="p1ps", name=f"pss{i}") for i in range(B)]
                    for k in range(4):
                        for b in range(4):
                            nc.tensor.matmul(
                                pss[b][:],
                                wih_sb[:, (k * 16 + mc) * 128:(k * 16 + mc + 1) * 128],
                                xT_sb[:, 4096 * k + 1024 * b + 512 * h2:
                                      4096 * k + 1024 * b + 512 * h2 + 512],
                                start=(k == 0), stop=(k == 3))
                    for b in range(4):
                        ev = p1e.tile([128, 512], BF16)
                        nc.scalar.activation(ev[:], pss[b][:], AF.Identity,
                                             bias=mb_sb[:, mc:mc + 1])
                        cc = 4 * mc + b
                        tp0 = (T + 2 * SPB) * cc + 512 * h2
                        nc.sync.dma_start(xwt[:, tp0:tp0 + 512], ev[:])

        # ================= Phase 2: recurrence ==============================
        with ExitStack() as p2:
            p2w = p2.enter_context(tc.tile_pool(name="p2w", bufs=1))
            whh_sb = p2w.tile([128, 4 * 16 * 128], BF16)
            nc.sync.dma_start(whh_sb[:], whhT_in[:, :])
            wpih_sb = p2w.tile([128, 4 * 4 * P], BF16)
            nc.sync.dma_start(wpih_sb[:], wpihT_in[:, :])
            wphh_sb = p2w.tile([128, 4 * P], BF16)
            nc.sync.dma_start(wphh_sb[0:P, :], wphhT_in[:, :])
            w3_sb = p2w.tile([128, 4], BF16)
            nc.sync.dma_start(w3_sb[0:P, :], w3T_in[:, :])
            bp_sb = p2w.tile([128, 80], BF16)
            nc.sync.dma_start(bp_sb[0:1, :], bp80_in[:, :])
            ones64 = p2w.tile([128, 4 * SPB], BF16)
            nc.vector.memset(ones64[0:1, :], 1.0)
            ident = p2w.tile([128, 128], BF16)
            from concourse.masks import make_identity
            make_identity(nc, ident[:])
            bm_sb = p2w.tile([128, 16], F32)
            nc.sync.dma_start(bm_sb[0:1, :], bm_in[:, :])
            invLp_sb = p2w.tile([128, 128], F32)
            nc.sync.dma_start(invLp_sb[0:1, :], invLp_in[:, :])
            jl_sb = p2w.tile([128, 4 * NST], F32)
            nc.sync.dma_start(jl_sb[0:1, :], jl4_in[:, :])

            astack = p2w.tile([128, 4 * NST], F32)
            bstack = p2w.tile([128, 4 * NST], F32)
            sigstack = p2w.tile([128, 4 * NST], F32)
            hpstack = p2w.tile([128, 4 * NPS], BF16)
            nc.vector.memset(hpstack[0:P, 0:4 * (SPB + 1)], 0.0)

            c_sb = p2w.tile([128, 16], F32)
            cp_sb = p2w.tile([128, 4], F32)
            h16 = p2w.tile([128, 16], BF16)
            hp16 = p2w.tile([128, 4], BF16)
            nc.vector.memset(c_sb[:], 0.0)
            nc.vector.memset(cp_sb[0:P, :], 0.0)
            nc.vector.memset(h16[:], 0.0)
            nc.vector.memset(hp16[0:P, :], 0.0)

            xwA = p2w.tile([128, SPB * 64], BF16)
            xwB = p2w.tile([128, SPB * 64], BF16)
            work = p2.enter_context(tc.tile_pool(name="work", bufs=2))
            gps_pool = p2.enter_context(tc.tile_pool(name="gps", bufs=2, space="PSUM"))
            pos_pool = p2.enter_context(tc.tile_pool(name="posps", bufs=2, space="PSUM"))
            w3_pool = p2.enter_context(tc.tile_pool(name="w3ps", bufs=2, space="PSUM"))

            xwt_v = xwt[:, :].rearrange("p (cc t) -> p cc t", cc=64)
            enc3 = encTbuf[:, :].rearrange("p (u x) -> p u x", x=16)

            # gate chunk order: g~ (12-15) first, i,f (0-7), o (8-11) last
            MC_ORDER = [12, 13, 14, 15, 0, 1, 2, 3, 4, 5, 6, 7, 8, 9, 10, 11]

            def pos_block(it):
                """Open the psum bank for block (it-1): bias + batched
                xp = Wp_ih @ h for its 16 steps. Col layout: 16*s + 4*g + b."""
                posb = pos_pool.tile([128, 16 * SPB], F32)
                posb3 = posb[0:P, :].rearrange("p (s x) -> p s x", x=16)
                for g in range(4):
                    nc.tensor.matmul(
                        posb3[:, :, 4 * g:4 * g + 4],
                        bp_sb[0:1, P * g:P * g + P],
                        ones64[0:1, :],
                        start=(g == 0), stop=False)
                for g in range(4):
                    for k in range(4):
                        nc.tensor.matmul(
                            posb3[:, :, 4 * g:4 * g + 4],
                            wpih_sb[:, 80 * k + P * g:80 * k + P * g + P],
                            enc3[:, bass.ds(SPB * it, SPB), 4 * k:4 * k + 4],
                            start=False, stop=False)
                return posb

            def pos_step(it, s, posb):
                for g in range(4):
                    nc.tensor.matmul(
                        posb[0:P, 16 * s + 4 * g:16 * s + 4 * g + 4],
                        wphh_sb[0:P, P * g:P * g + P],
                        hp16[0:P, :],
                        start=False, stop=(s == SPB - 1 and g == 3))
                sp = work.tile([128, 12], F32)
                nc.scalar.activation(sp[0:P, :], posb[0:P, 16 * s:16 * s + 12],
                                     AF.Sigmoid)
                tp = work.tile([128, 4], F32)
                nc.scalar.activation(tp[0:P, :], posb[0:P, 16 * s + 12:16 * s + 16],
                                     AF.Tanh)
                u1 = work.tile([128, 4], F32)
                nc.vector.tensor_mul(u1[0:P, :], sp[0:P, 4:8], cp_sb[0:P, :])
                u2 = work.tile([128, 4], F32)
                nc.vector.tensor_mul(u2[0:P, :], sp[0:P, 0:4], tp[0:P, :])
                nc.vector.tensor_add(cp_sb[0:P, :], u1[0:P, :], u2[0:P, :])
                tcp = work.tile([128, 4], F32)
                nc.scalar.activation(tcp[0:P, :], cp_sb[0:P, :], AF.Tanh)
                nc.vector.tensor_mul(hp16[0:P, :], sp[0:P, 8:12], tcp[0:P, :])
                nc.vector.tensor_copy(
                    hpstack[0:P, bass.ds(4 * SPB * it + 4 * s + 4, 4)], hp16[0:P, :])

            def w3_block(it):
                """W3/mw/sigma batch for block it-1 -> stacks at ds(64*it)."""
                w3p = w3_pool.tile([128, 16 * SPB], F32)
                W = 4 * SPB
                for r in range(4):
                    nc.tensor.matmul(
                        w3p[0:1, W * r:W * r + W],
                        w3_sb[0:P, r:r + 1],
                        hpstack[0:P, bass.ds(4 * SPB * it + 4, W)],
                        start=(r == 0), stop=(r == 3))
                nc.scalar.activation(astack[0:1, bass.ds(W * it, W)],
                                     w3p[0:1, 0:W], AF.Relu,
                                     bias=bm_sb[0:1, 0:1])
                w1t = work.tile([128, W], F32, tag="w1t")
                nc.scalar.activation(w1t[0:1, :], w3p[0:1, W:2 * W], AF.Relu,
                                     bias=bm_sb[0:1, 4:5])
                w2t = work.tile([128, W], F32, tag="w2t")
                nc.scalar.activation(w2t[0:1, :], w3p[0:1, 2 * W:3 * W], AF.Relu,
                                     bias=bm_sb[0:1, 8:9])
                nc.scalar.activation(sigstack[0:1, bass.ds(W * it, W)],
                                     w3p[0:1, 3 * W:4 * W], AF.Sigmoid,
                                     bias=bm_sb[0:1, 12:13])
                z1 = work.tile([128, W], F32, tag="z1")
                nc.vector.tensor_mul(z1[0:1, :], w2t[0:1, :],
                                     jl_sb[0:1, bass.ds(W * it, W)])
                z2 = work.tile([128, W], F32, tag="z2")
                nc.vector.tensor_mul(z2[0:1, :], w1t[0:1, :], invLp_sb[0:1, 0:W])
                nc.vector.tensor_add(bstack[0:1, bass.ds(W * it, W)],
                                     z1[0:1, :], z2[0:1, :])

            def main_step(it, s, xw_v):
                g_ps = gps_pool.tile([128, 64], F32)
                # xw injection seeds the accumulation group (identity matmul)
                nc.tensor.matmul(g_ps[:, 0:64], ident[:, :], xw_v[:, :, s],
                                 start=True, stop=False)
                for i, mc in enumerate(MC_ORDER):
                    for k in range(4):
                        nc.tensor.matmul(
                            g_ps[:, 4 * mc:4 * mc + 4],
                            whh_sb[:, (k * 16 + mc) * 128:(k * 16 + mc + 1) * 128],
                            h16[:, 4 * k:4 * k + 4],
                            start=False,
                            stop=(i == 15 and k == 3))
                    if i == 3:  # g~ chunks done
                        tg = work.tile([128, 16], F32, tag="tg")
                        nc.scalar.activation(tg[:], g_ps[:, 48:64], AF.Tanh)
                    elif i == 11:  # i,f chunks done
                        sif = work.tile([128, 32], F32, tag="sif")
                        nc.scalar.activation(sif[:], g_ps[:, 0:32], AF.Sigmoid)
                        t2 = work.tile([128, 16], F32, tag="t2")
                        nc.vector.tensor_mul(t2[:], sif[:, 0:16], tg[:])
                        t1 = work.tile([128, 16], F32, tag="t1")
                        nc.vector.tensor_mul(t1[:], sif[:, 16:32], c_sb[:])
                        nc.vector.tensor_add(c_sb[:], t1[:], t2[:])
                        tct = work.tile([128, 16], F32, tag="tct")
                        nc.scalar.activation(tct[:], c_sb[:], AF.Tanh)
                # o chunks done
                so = work.tile([128, 16], F32, tag="so")
                nc.scalar.activation(so[:], g_ps[:, 32:48], AF.Sigmoid)
                nc.vector.tensor_mul(h16[:], so[:], tct[:])
                nc.vector.tensor_copy(
                    encTbuf[:, bass.ds(16 * SPB * it + 16 * s + 16 * SPB, 16)], h16[:])

            xwA_v = xwA[:].rearrange("p (cc t) -> p cc t", cc=64)
            xwB_v = xwB[:].rearrange("p (cc t) -> p cc t", cc=64)
            nc.sync.dma_start(xwA_v, xwt_v[:, :, 0:SPB])
            nc.sync.dma_start(xwB_v, xwt_v[:, :, SPB:2 * SPB])
            with tc.For_i(0, NBLK // 2) as it:
                for j, xw_v in ((0, xwA_v), (1, xwB_v)):
                    bi = 2 * it + j
                    posb = pos_block(bi)
                    for s in range(SPB):
                        main_step(bi, s, xw_v)
                        pos_step(bi, s, posb)
                    w3_block(bi)
                    # prefetch this buffer's next block (2 blocks ahead);
                    # overlaps the other buffer's compute
                    nc.sync.dma_start(
                        xw_v, xwt_v[:, :, bass.ds((2 * it + j + 2) * SPB, SPB)])

            # epilogue: positional block 63
            posb = pos_block(NBLK)
            for s in range(SPB):
                pos_step(NBLK, s, posb)
            w3_block(NBLK)

            # den = 1/(2*sigma^2 + eps); mu via scan (bf16 outputs feed the
            # attention broadcast matmuls; scan state itself stays fp32)
            with nc.allow_low_precision(reason="mu/den stacks in bf16 for 2x DVE and 1cyc/row matmul broadcast"):
                nc.vector.tensor_mul(denstack[0:1, :], sigstack[0:1, 4 * SPB:4 * SPB + 4 * T],
                                     sigstack[0:1, 4 * SPB:4 * SPB + 4 * T])
                nc.vector.tensor_scalar(denstack[0:1, :], denstack[0:1, :],
                                        2.0, EPS_SIG,
                                        mybir.AluOpType.mult, mybir.AluOpType.add)
                nc.vector.reciprocal(denstack[0:1, :], denstack[0:1, :])
                a_v = astack[0:1, 4 * SPB:4 * SPB + 4 * T].rearrange("o (t b) -> o t b", b=4)
                b_v = bstack[0:1, 4 * SPB:4 * SPB + 4 * T].rearrange("o (t b) -> o t b", b=4)
                m_v = mustack[0:1, :].rearrange("o (t b) -> o t b", b=4)
                for b in range(4):
                    nc.vector.tensor_tensor_scan(
                        m_v[:, :, b], a_v[:, :, b], b_v[:, :, b], 0.0,
                        mybir.AluOpType.mult, mybir.AluOpType.add)

        encT_v = encTbuf[:, 16 * SPB:16 * SPB + 16 * T].rearrange("p (t x) -> p t x", x=16)
        mu_v = mustack[0:1, :].rearrange("o (t b) -> o t b", b=4)
        den_v = denstack[0:1, :].rearrange("o (t b) -> o t b", b=4)

        ctx_pool = ctx.enter_context(tc.tile_pool(name="ctxp", bufs=1))
        ctxTs = [ctx_pool.tile([128, 4 * T], BF16, tag=f"ctxT{b}", name=f"ctxT{b}") for b in range(B)]

        # ================= Phase 3a: attention ==============================
        with ExitStack() as p3:
            cpool = p3.enter_context(tc.tile_pool(name="p3c", bufs=1))
            relM_sb = cpool.tile([128, 8 * T], BF16)
            nc.sync.dma_start(relM_sb[:], relM_in[:, :])
            ident = cpool.tile([128, 128], BF16)
            from concourse.masks import make_identity
            make_identity(nc, ident[:])
            ones_col = cpool.tile([128, 1], BF16)
            nc.vector.memset(ones_col[:], 1.0)
            ones_row = cpool.tile([128, 128], BF16)
            nc.vector.memset(ones_row[0:1, :], 1.0)

            bpool = p3.enter_context(tc.tile_pool(name="p3b", bufs=1))
            wk = p3.enter_context(tc.tile_pool(name="p3wk", bufs=2))
            nrm = p3.enter_context(tc.tile_pool(name="p3n", bufs=1))
            tps_pool = p3.enter_context(tc.tile_pool(name="tpsp", bufs=2, space="PSUM"))
            ps512 = p3.enter_context(tc.tile_pool(name="ps512", bufs=2, space="PSUM"))
            rowps = p3.enter_context(tc.tile_pool(name="rowps", bufs=2, space="PSUM"))

            for b in range(B):
                muB = bpool.tile([128, T], BF16, tag="muB")
                dnB = bpool.tile([128, T], BF16, tag="dnB")
                rcB = bpool.tile([128, T], BF16, tag="rcB")
                for half in range(2):
                    mps = rowps.tile([128, 512], F32, tag="mps")
                    nc.tensor.matmul(mps[:], ones_row[0:1, :],
                                     mu_v[:, 512 * half:512 * half + 512, b],
                                     start=True, stop=True)
                    nc.scalar.copy(muB[:, 512 * half:512 * half + 512], mps[:])
                    dps = rowps.tile([128, 512], F32, tag="mps")
                    nc.tensor.matmul(dps[:], ones_row[0:1, :],
                                     den_v[:, 512 * half:512 * half + 512, b],
                                     start=True, stop=True)
                    nc.scalar.copy(dnB[:, 512 * half:512 * half + 512], dps[:])

                wstack = bpool.tile([128, 8 * T], BF16, tag="wstack")
                for tt in range(8):
                    d0 = wk.tile([128, T], BF16, tag="d0")
                    nc.vector.tensor_sub(d0[:], relM_sb[:, T * tt:T * tt + T], muB[:])
                    nc.vector.tensor_mul(d0[:], d0[:], d0[:])
                    nc.vector.tensor_mul(d0[:], d0[:], dnB[:])
                    nc.scalar.activation(wstack[:, T * tt:T * tt + T], d0[:],
                                         AF.Exp, scale=-1.0)
                wsmax = nrm.tile([128, T], F32, tag="wsmax")
                wsb = nrm.tile([128, T], BF16, tag="wsb")
                for half in range(2):
                    wps = rowps.tile([128, 512], F32, tag="mps")
                    for tt in range(8):
                        nc.tensor.matmul(
                            wps[0:1, :], ones_col[:, 0:1],
                            wstack[:, T * tt + 512 * half:T * tt + 512 * half + 512],
                            start=(tt == 0), stop=(tt == 7))
                    nc.vector.tensor_scalar_max(
                        wsmax[0:1, 512 * half:512 * half + 512], wps[0:1, :],
                        EPS_NORM)
                nc.vector.reciprocal(wsmax[0:1, :], wsmax[0:1, :])
                nc.vector.tensor_copy(wsb[0:1, :], wsmax[0:1, :])
                for half in range(2):
                    rps = rowps.tile([128, 512], F32, tag="mps")
                    nc.tensor.matmul(rps[:], ones_row[0:1, :],
                                     wsb[0:1, 512 * half:512 * half + 512],
                                     start=True, stop=True)
                    nc.scalar.copy(rcB[:, 512 * half:512 * half + 512], rps[:])

                encnat = bpool.tile([128, 8 * 512], BF16, tag="encnat")
                for tt in range(8):
                    for c in range(4):
                        tps = tps_pool.tile([128, 128], BF16)
                        nc.tensor.transpose(
                            tps[:], encT_v[:, 128 * tt:128 * tt + 128, 4 * c + b],
                            ident[:])
                        nc.scalar.copy(
                            encnat[:, 512 * tt + 128 * c:512 * tt + 128 * c + 128],
                            tps[:])

                for hc in range(4):
                    for half in range(2):
                        cps = ps512.tile([128, 512], F32)
                        for tt in range(8):
                            nc.tensor.matmul(
                                cps[:],
                                encnat[:, 512 * tt + 128 * hc:512 * tt + 128 * hc + 128],
                                wstack[:, T * tt + 512 * half:T * tt + 512 * half + 512],
                                start=(tt == 0), stop=(tt == 7))
                        nc.vector.tensor_mul(
                            ctxTs[b][:, T * hc + 512 * half:T * hc + 512 * half + 512],
                            cps[:], rcB[:, 512 * half:512 * half + 512])

        # ================= Phase 3b: combined + decoder =====================
        with ExitStack() as p4:
            c4 = p4.enter_context(tc.tile_pool(name="p4c", bufs=1))
            wc_sb = c4.tile([128, 8 * 4 * 128], BF16)
            nc.sync.dma_start(wc_sb[:], wcT_in[:, :])
            bc_sb = c4.tile([128, 4], F32)
            nc.sync.dma_start(bc_sb[:], bc_in[:, :])
            emb_sb = c4.tile([128, 4 * VSH], BF16)
            nc.sync.dma_start(emb_sb[:], embT_in[:, :])
            bwork = p4.enter_context(tc.tile_pool(name="p4b", bufs=1))
            dec_e = p4.enter_context(tc.tile_pool(name="p4d", bufs=4))
            qps_pool = p4.enter_context(tc.tile_pool(name="qps", bufs=3, space="PSUM"))

            for b in range(B):
                combT = bwork.tile([128, 4 * T], BF16, tag="combT")
                for m in range(4):
                    for half in range(2):
                        qps = qps_pool.tile([128, 512], F32, tag="q")
                        for k in range(8):
                            if k < 4:
                                rhs = ctxTs[b][:, T * k + 512 * half:
                                               T * k + 512 * half + 512]
                            else:
                                rhs = encT_v[:, 512 * half:512 * half + 512,
                                             4 * (k - 4) + b]
                            nc.tensor.matmul(
                                qps[:],
                                wc_sb[:, (k * 4 + m) * 128:(k * 4 + m + 1) * 128],
                                rhs, start=(k == 0), stop=(k == 7))
                        nc.scalar.activation(
                            combT[:, T * m + 512 * half:T * m + 512 * half + 512],
                            qps[:], AF.Tanh, bias=bc_sb[:, m:m + 1])

                for tc8 in range(8):
                    for vc in range(8):
                        dps = qps_pool.tile([128, 500], F32, tag="q")
                        for k in range(4):
                            nc.tensor.matmul(
                                dps[:],
                                combT[:, T * k + 128 * tc8:T * k + 128 * tc8 + 128],
                                emb_sb[:, VSH * k + 500 * vc:VSH * k + 500 * vc + 500],
                                start=(k == 0), stop=(k == 3))
                        oe = dec_e.tile([128, 500], BF16, tag="oe")
                        nc.scalar.copy(oe[:], dps[:])
                        nc.sync.dma_start(
                            logits_out[T * b + 128 * tc8:T * b + 128 * tc8 + 128,
                                       500 * vc:500 * vc + 500],
                            oe[:])

    nc.finalize()
    return nc


_NC_CACHE = [None]


def _get_nc():
    if _NC_CACHE[0] is None:
        _NC_CACHE[0] = build_nc()
    return _NC_CACHE[0]


def kernel(input_ids, pad_lengths, emb, dec_bias, Wih, Whh, bih, bhh,
           Wp_ih, Wp_hh, bp_ih, bp_hh, Wmu, bmu, Wsig, bsig, Wc, bc):
    input_ids = np.asarray(input_ids)
    pad_lengths = np.asarray(pad_lengths)
    emb = _f32(emb); dec_bias = _f32(dec_bias)
    Wih = _f32(Wih); Whh = _f32(Whh); bih = _f32(bih); bhh = _f32(bhh)
    Wp_ih = _f32(Wp_ih); Wp_hh = _f32(Wp_hh); bp_ih = _f32(bp_ih); bp_hh = _f32(bp_hh)
    Wmu = _f32(Wmu); bmu = _f32(bmu); Wsig = _f32(Wsig); bsig = _f32(bsig)
    Wc = _f32(Wc); bc = _f32(bc)

    perm = np.r_[0:H, H:2 * H, 3 * H:4 * H, 2 * H:3 * H]
    permp = np.r_[0:P, P:2 * P, 3 * P:4 * P, 2 * P:3 * P]

    x = emb[input_ids]                                   # [B,T,H]
    xT = x.reshape(B, T, 4, 128).transpose(3, 2, 0, 1).reshape(128, 4 * B * T)

    def pack_kxm(Wt, nk, nm):
        return Wt.reshape(nk, 128, nm, 128).transpose(1, 0, 2, 3).reshape(
            128, nk * nm * 128)

    wihT = pack_kxm(Wih[perm].T, 4, 16)
    whhT = pack_kxm(Whh[perm].T, 4, 16)
    mbias = (bih + bhh)[perm].reshape(16, 128).T

    wpihT = Wp_ih[permp].reshape(4, P, 4, 128).transpose(3, 2, 0, 1).reshape(
        128, 4 * 4 * P)
    wphhT = Wp_hh[permp].T                               # [20, 80]
    w3T = np.vstack([Wmu, Wsig]).T                       # [20, 4]
    bpv = (bp_ih + bp_hh)[permp]
    bp80 = bpv.reshape(1, 80)
    bm4 = np.concatenate([bmu, bsig])
    bm_t = np.repeat(bm4[:, None], 4, axis=1).reshape(1, 16)

    invL = (1.0 / pad_lengths.astype(np.float64)).astype(np.float32)  # [4]
    invLp = np.tile(invL, 32).reshape(1, 128)
    jl4 = np.zeros((1, 4 * (32 + T)), np.float32)
    tgrid = np.arange(T, dtype=np.float64) + 1.0
    jl4[0, 128:] = (tgrid[:, None] * invL.astype(np.float64)[None, :]).astype(
        np.float32).reshape(-1)

    ti = np.arange(T, dtype=np.float64)
    relM = (ti[:, None] / (ti[None, :] + 1.0)).astype(np.float32)
    relM[ti[:, None] > ti[None, :]] = 1e9
    relM_p = relM.reshape(8, 128, T).transpose(1, 0, 2).reshape(128, 8 * T)

    wcT = Wc.reshape(4, 128, 8, 128).transpose(3, 2, 0, 1).reshape(128, 8 * 4 * 128)
    bc_t = bc.reshape(4, 128).T

    common = {
        "xT": _bf(xT), "wihT": _bf(wihT), "whhT": _bf(whhT),
        "mbias": _f32(mbias), "wpihT": _bf(wpihT), "wphhT": _bf(wphhT),
        "w3T": _bf(w3T), "bp80": _bf(bp80), "bm": _f32(bm_t),
        "invLp": _f32(invLp), "jl4": _f32(jl4), "relM": _bf(relM_p),
        "wcT": _bf(wcT), "bc": _f32(bc_t),
    }
    in_maps = []
    for c in range(NCORES):
        sh = emb[VSH * c:VSH * (c + 1)]
        embT = sh.reshape(VSH, 4, 128).transpose(2, 1, 0).reshape(128, 4 * VSH)
        m = dict(common)
        m["embT"] = _bf(embT)
        in_maps.append(m)

    nc = _get_nc()
    trace = bool(os.environ.get("KERNEL_TRACE"))
    res = run_bass_kernel_spmd(nc, in_maps, core_ids=list(range(NCORES)),
                               trace=trace)
    LAST_EXEC_NS[0] = res.exec_time_ns

    parts = [res.results[c]["logits"].reshape(B, T, VSH) for c in range(NCORES)]
    logits = np.concatenate(parts, axis=-1).astype(np.float32)
    if np.any(dec_bias):
        logits = logits + dec_bias
    return logits
